# revision 1
# baseline (speedup 1.0000x reference)
"""Trainium2 Bass kernel for nn_CMFA (dense_transformer, seq_len=1 cross-attention).

Math notes (exact simplifications vs the reference):
  - softmax over a single key is exactly 1.0, so the attention output is
    exactly the v-projection: mha(q,k,v) = (v @ Wv.T + bv) @ Wo.T + bo.
    The q/k projections never influence the output.
  - Wv -> Wo -> fi2 is a linear chain (no nonlinearity), so it is folded on
    the host:  V = [v1, i_] @ Wcat.T + bcat  with
      Wcat = [fi2 @ (Wo @ Wv), fi2],  bcat = fi2 @ (Wo @ bv + bo) + fi2_b
    (the i_ column block carries the residual through fi2).

Device layout: activations are feature-major ("transposed", [feat, batch]) so
every matmul contracts over the partition dim and every DMA is contiguous.
The host pre-transposes the batch shards of i/t and transposes the output
back. Pure data parallel across 8 cores; weights replicated.

Per-(layer, k-chunk) weight tiles give exact DMA->matmul dependencies, so
the PE starts as soon as the first 256KB chunks land. Input loads for batch
tile n+1 are emitted right after tile n's fi1 matmuls (with a 16-slot x
pool) so the in-order Sync dispatch queue prefetches them ahead of tile n's
output stores.
"""

import numpy as np

B, IMG, TAB, HID = 32768, 2048, 128, 512
NCORES = 8
BS = B // NCORES  # rows per core
NT = 512          # batch-tile (matmul moving/free dim)

_CACHE = {}


def _pack_blocks(WT: np.ndarray, K: int, M: int) -> np.ndarray:
    """[K*128, M*128] -> [128, K*M*128] with col ((k*M+m)*128 + j) = WT[k*128+p, m*128+j]."""
    out = WT.reshape(K, 128, M, 128).transpose(1, 0, 2, 3).reshape(128, K * M * 128)
    return np.ascontiguousarray(out, dtype=np.float32)


def _build_nc(bs: int):
    import concourse.bass as bass
    import concourse.tile as tile
    from concourse import bacc, mybir

    f32 = mybir.dt.float32
    f32r = mybir.dt.float32r
    Relu = mybir.ActivationFunctionType.Relu
    Ident = mybir.ActivationFunctionType.Identity
    ntiles = bs // NT

    nc = bacc.Bacc("TRN2", target_bir_lowering=False, debug=False)

    iT_d = nc.dram_tensor("iT", [IMG, bs], f32r, kind="ExternalInput").ap()
    tT_d = nc.dram_tensor("tT", [TAB, bs], f32r, kind="ExternalInput").ap()
    w_fi1_d = nc.dram_tensor("w_fi1", [128, 64 * 128], f32r, kind="ExternalInput").ap()
    w_ft1_d = nc.dram_tensor("w_ft1", [128, 4 * 128], f32r, kind="ExternalInput").ap()
    w_ci1_d = nc.dram_tensor("w_ci1", [128, 16 * 128], f32r, kind="ExternalInput").ap()
    w_ct1_d = nc.dram_tensor("w_ct1", [128, 16 * 128], f32r, kind="ExternalInput").ap()
    w_V_d = nc.dram_tensor("w_V", [128, 32 * 128], f32r, kind="ExternalInput").ap()
    w_T_d = nc.dram_tensor("w_T", [128, 32 * 128], f32r, kind="ExternalInput").ap()
    bias_d = nc.dram_tensor("bias", [128, 24], f32, kind="ExternalInput").ap()
    out_d = nc.dram_tensor("outT", [2 * HID, bs], f32, kind="ExternalOutput").ap()

    with tile.TileContext(nc) as tc:
        with (
            tc.tile_pool(name="w", bufs=1) as wpool,
            tc.tile_pool(name="x", bufs=16) as xpool,
            tc.tile_pool(name="h", bufs=6) as hpool,
            tc.tile_pool(name="o", bufs=8) as opool,
            tc.tile_pool(name="ps", bufs=8, space="PSUM") as pspool,
        ):
            def wchunks(K, lname):
                return [wpool.tile([128, 4 * 128], f32r, name=f"w_{lname}_{k}")
                        for k in range(K)]

            wf1 = wchunks(16, "fi1")
            wt1 = wchunks(1, "ft1")
            wc1 = wchunks(4, "ci1")
            wc2 = wchunks(4, "ct1")
            wV = wchunks(8, "V")
            wT = wchunks(8, "T")
            bt = wpool.tile([128, 24], f32, name="bias_t")

            def xload(n):
                xs = []
                c0 = n * NT
                for k in range(16):
                    xk = xpool.tile([128, NT], f32r, tag="x", name=f"xk_{n}_{k}")
                    nc.sync.dma_start(xk[:], iT_d[128 * k:128 * (k + 1), c0:c0 + NT])
                    xs.append(xk)
                return xs

            # preamble: first tile's x chunks interleaved with fi1 weight chunks
            x_cur = [xpool.tile([128, NT], f32r, tag="x", name=f"xk_0_{k}")
                     for k in range(16)]
            nc.sync.dma_start(bt[:], bias_d[:])
            for k in range(16):
                nc.sync.dma_start(x_cur[k][:], iT_d[128 * k:128 * (k + 1), 0:NT])
                nc.sync.dma_start(wf1[k][:], w_fi1_d[:, 512 * k:512 * (k + 1)])
            xt_cur = xpool.tile([128, NT], f32r, tag="xt", bufs=2, name="xt_0")
            nc.sync.dma_start(xt_cur[:], tT_d[:, 0:NT])
            for tiles, dram in [(wt1, w_ft1_d), (wc1, w_ci1_d), (wc2, w_ct1_d),
                                (wV, w_V_d), (wT, w_T_d)]:
                for j, wtile in enumerate(tiles):
                    nc.sync.dma_start(wtile[:], dram[:, 512 * j:512 * (j + 1)])

            def mm(ps_ap, wtiles, k, m, x_ap, start, stop):
                nc.tensor.matmul(
                    ps_ap,
                    wtiles[k][:, m * 128:(m + 1) * 128],
                    x_ap,
                    start=start,
                    stop=stop,
                )

            for n in range(ntiles):
                c0 = n * NT
                # ---- i_ = relu(i @ fi1.T + b) ----
                ps1 = [pspool.tile([128, NT], f32, tag="ps", name=f"ps1_{n}_{_m}") for _m in range(4)]
                for k in range(16):
                    for m in range(4):
                        mm(ps1[m][:], wf1, k, m, x_cur[k][:], k == 0, k == 15)

                # prefetch next tile's inputs (early in Sync program order)
                if n + 1 < ntiles:
                    x_nxt = xload(n + 1)
                    xt_nxt = xpool.tile([128, NT], f32r, tag="xt", bufs=2,
                                        name=f"xt_{n + 1}")
                    nc.sync.dma_start(xt_nxt[:], tT_d[:, c0 + NT:c0 + 2 * NT])

                i_ = [hpool.tile([128, NT], f32r, tag="i_", name=f"i__{n}_{_m}") for _m in range(4)]
                for m in range(4):
                    nc.scalar.activation(i_[m][:], ps1[m][:], Relu, bias=bt[:, m:m + 1])

                # ---- t_ = relu(t @ ft1.T + b) ----
                ps2 = [pspool.tile([128, NT], f32, tag="ps", name=f"ps2_{n}_{_m}") for _m in range(4)]
                for m in range(4):
                    mm(ps2[m][:], wt1, 0, m, xt_cur[:], True, True)
                t_ = [hpool.tile([128, NT], f32r, tag="t_", name=f"t__{n}_{_m}") for _m in range(4)]
                for m in range(4):
                    nc.scalar.activation(t_[m][:], ps2[m][:], Relu, bias=bt[:, 4 + m:5 + m])

                # ---- v1 = relu(i_ @ ci1.T + b) ----
                ps3 = [pspool.tile([128, NT], f32, tag="ps", name=f"ps3_{n}_{_m}") for _m in range(4)]
                for k in range(4):
                    for m in range(4):
                        mm(ps3[m][:], wc1, k, m, i_[k][:], k == 0, k == 3)
                v1 = [hpool.tile([128, NT], f32r, tag="v1", name=f"v1_{n}_{_m}") for _m in range(4)]
                for m in range(4):
                    nc.scalar.activation(v1[m][:], ps3[m][:], Relu, bias=bt[:, 8 + m:9 + m])

                # ---- v2 = relu(t_ @ ct1.T + b) ----
                ps4 = [pspool.tile([128, NT], f32, tag="ps", name=f"ps4_{n}_{_m}") for _m in range(4)]
                for k in range(4):
                    for m in range(4):
                        mm(ps4[m][:], wc2, k, m, t_[k][:], k == 0, k == 3)
                v2 = [hpool.tile([128, NT], f32r, tag="v2", name=f"v2_{n}_{_m}") for _m in range(4)]
                for m in range(4):
                    nc.scalar.activation(v2[m][:], ps4[m][:], Relu, bias=bt[:, 12 + m:13 + m])

                # ---- V = [v1, i_] @ WcatV.T + bcatV ----
                psV = [pspool.tile([128, NT], f32, tag="ps", name=f"psV_{n}_{_m}") for _m in range(4)]
                for k in range(4):
                    for m in range(4):
                        mm(psV[m][:], wV, k, m, v1[k][:], k == 0, False)
                for k in range(4):
                    for m in range(4):
                        mm(psV[m][:], wV, 4 + k, m, i_[k][:], False, k == 3)
                for m in range(4):
                    oV = opool.tile([128, NT], f32, tag="o", name=f"oV_{n}_{m}")
                    nc.scalar.activation(oV[:], psV[m][:], Ident, bias=bt[:, 16 + m:17 + m])
                    nc.sync.dma_start(out_d[128 * m:128 * (m + 1), c0:c0 + NT], oV[:])

                # ---- T = [v2, t_] @ WcatT.T + bcatT ----
                psT = [pspool.tile([128, NT], f32, tag="ps", name=f"psT_{n}_{_m}") for _m in range(4)]
                for k in range(4):
                    for m in range(4):
                        mm(psT[m][:], wT, k, m, v2[k][:], k == 0, False)
                for k in range(4):
                    for m in range(4):
                        mm(psT[m][:], wT, 4 + k, m, t_[k][:], False, k == 3)
                for m in range(4):
                    oT = opool.tile([128, NT], f32, tag="o", name=f"oT_{n}_{m}")
                    nc.scalar.activation(oT[:], psT[m][:], Ident, bias=bt[:, 20 + m:21 + m])
                    nc.sync.dma_start(
                        out_d[HID + 128 * m:HID + 128 * (m + 1), c0:c0 + NT], oT[:]
                    )

                if n + 1 < ntiles:
                    x_cur = x_nxt
                    xt_cur = xt_nxt

    nc.compile()
    return nc


def _host_pack(inp: dict):
    f8 = np.float64
    fi1_w, fi1_b = inp["fi1_w"], inp["fi1_b"]
    ft1_w, ft1_b = inp["ft1_w"], inp["ft1_b"]
    ci1_w, ci1_b = inp["ci1_w"], inp["ci1_b"]
    ct1_w, ct1_b = inp["ct1_w"], inp["ct1_b"]

    def fold(wv, bv, wo, bo, f_w, f_b):
        Wvo = wo.astype(f8) @ wv.astype(f8)
        bvo = wo.astype(f8) @ bv.astype(f8) + bo.astype(f8)
        Wcat = np.concatenate([f_w.astype(f8) @ Wvo, f_w.astype(f8)], axis=1)
        bcat = f_w.astype(f8) @ bvo + f_b.astype(f8)
        return Wcat.astype(np.float32), bcat.astype(np.float32)

    WcatV, bcatV = fold(inp["aV_wv"], inp["aV_bv"], inp["aV_wo"], inp["aV_bo"],
                        inp["fi2_w"], inp["fi2_b"])
    WcatT, bcatT = fold(inp["aT_wv"], inp["aT_bv"], inp["aT_wo"], inp["aT_bo"],
                        inp["ft2_w"], inp["ft2_b"])

    weights = {
        "w_fi1": _pack_blocks(np.ascontiguousarray(fi1_w.T), 16, 4),
        "w_ft1": _pack_blocks(np.ascontiguousarray(ft1_w.T), 1, 4),
        "w_ci1": _pack_blocks(np.ascontiguousarray(ci1_w.T), 4, 4),
        "w_ct1": _pack_blocks(np.ascontiguousarray(ct1_w.T), 4, 4),
        "w_V": _pack_blocks(np.ascontiguousarray(WcatV.T), 8, 4),
        "w_T": _pack_blocks(np.ascontiguousarray(WcatT.T), 8, 4),
    }
    cols = []
    for b in (fi1_b, ft1_b, ci1_b, ct1_b, bcatV, bcatT):
        for m in range(4):
            cols.append(b[128 * m:128 * (m + 1)])
    weights["bias"] = np.ascontiguousarray(np.stack(cols, axis=1), dtype=np.float32)
    return weights


def kernel(**inputs) -> np.ndarray:
    from concourse import bass_utils

    i = np.asarray(inputs["i"], dtype=np.float32)
    t = np.asarray(inputs["t"], dtype=np.float32)
    weights = _host_pack(inputs)

    if "nc" not in _CACHE:
        _CACHE["nc"] = _build_nc(BS)
    nc = _CACHE["nc"]

    in_maps = []
    for c in range(NCORES):
        sl = slice(c * BS, (c + 1) * BS)
        m = dict(weights)
        m["iT"] = np.ascontiguousarray(i[sl].T)
        m["tT"] = np.ascontiguousarray(t[sl].T)
        in_maps.append(m)

    res = bass_utils.run_bass_kernel_spmd(nc, in_maps, core_ids=list(range(NCORES)))

    out = np.empty((B, 2 * HID), dtype=np.float32)
    for c in range(NCORES):
        out[c * BS:(c + 1) * BS] = res.results[c]["outT"].T
    return out



# revision 2
# speedup vs baseline: 1.2672x; 1.2672x over previous
"""Trainium2 Bass kernel for nn_CMFA (dense_transformer, seq_len=1 cross-attention).

Math notes (exact simplifications vs the reference):
  - softmax over a single key is exactly 1.0, so the attention output is
    exactly the v-projection: mha(q,k,v) = (v @ Wv.T + bv) @ Wo.T + bo.
    The q/k projections never influence the output.
  - Wv -> Wo -> fi2 is a linear chain (no nonlinearity), folded on the host:
      V = v1 @ (fi2 @ Wo @ Wv).T + i_ @ fi2.T + bcatV
    (the i_ term carries the residual through fi2), likewise for T.

Precision plan (validated on host against the fp64 reference; gate 2e-2):
  - fi1/ft1 and the residual halves (fi2/ft2) run in bf16 (~4e-3 end to end).
  - The attention-value paths (ci1, ct1 and the v1/v2 halves of the folded
    output matmuls) run in fp8e4m3 with DoubleRow perf mode: 2 contraction
    chunks per matmul at 2 MACs/cycle. Their signal contribution is small
    (wv/wo are 0.02-scale), so fp8 error lands at ~5.6e-3 total.
  - fp8 operands are pre-scaled by powers of two to dodge e4m3 subnormals;
    the bf16 residual weights are scaled by the same 2^17 group factor so
    both halves share one PSUM accumulation group, and a single output
    activation applies 2^-17 + bias.

Device layout: activations are feature-major ([feat, batch]); pure data
parallel across 8 cores, weights replicated. Batch tiles of NT=512 with
m-outer fi1 loops so activations pipeline behind the PE, and fp8 DoubleRow
matmuls interleaved between bf16 matmuls so their slow (no-FWL, 256-col)
LDWEIGHTS hide under bf16 matmul streaming.
"""

import numpy as np
import ml_dtypes

B, IMG, TAB, HID = 32768, 2048, 128, 512
NCORES = 8
BS = B // NCORES  # rows per core
NT = 512          # batch-tile (matmul moving/free dim)

# fp8 scaling (powers of two; e4m3 max-normal on TRN is 240)
S_I8 = 8.0        # i_ activation scale for the ci1 rhs
S_T8 = 16.0       # t_ activation scale for the ct1 rhs
S_V1 = 32.0       # v1 fp8 activation scale
S_V2 = 32.0
S_W8 = 1024.0     # ci1/ct1 weight scale
S_WV = 4096.0     # Vv/Tv (folded) weight scale
GRP = S_V1 * S_WV  # 2^17: shared psum group scale for the output matmuls

_CACHE = {}

_bf16 = ml_dtypes.bfloat16
_f8 = ml_dtypes.float8_e4m3  # TRN-style e4m3 (max 240)


def _pack_blocks(WT: np.ndarray, K: int, M: int) -> np.ndarray:
    """[K*128, M*128] -> [128, K*M*128] bf16, block (k,m) at cols (k*M+m)*128."""
    out = WT.reshape(K, 128, M, 128).transpose(1, 0, 2, 3).reshape(128, K * M * 128)
    return np.ascontiguousarray(out).astype(_bf16)


def _pack_dr(WT: np.ndarray, scale: float) -> np.ndarray:
    """[512, 512] -> [128, 16, 128] fp8 for DoubleRow: dim1 = (k2*4+m)*2+kk,
    value = scale*WT[(2*k2+kk)*128+p, m*128+j]."""
    w = (WT * scale).reshape(2, 2, 128, 4, 128)           # k2, kk, p, m, j
    w = w.transpose(2, 0, 3, 1, 4).reshape(128, 16, 128)  # p, (k2,m,kk), j
    return np.clip(np.ascontiguousarray(w), -240, 240).astype(_f8)


def _build_nc(bs: int):
    import concourse.bass as bass  # noqa: F401
    import concourse.tile as tile
    from concourse import bacc, mybir

    f32 = mybir.dt.float32
    bf = mybir.dt.bfloat16
    f8 = mybir.dt.float8e4
    DR = mybir.MatmulPerfMode.DoubleRow
    Relu = mybir.ActivationFunctionType.Relu
    Ident = mybir.ActivationFunctionType.Identity
    ntiles = bs // NT

    nc = bacc.Bacc("TRN2", target_bir_lowering=False, debug=False)

    iT_d = nc.dram_tensor("iT", [IMG, bs], bf, kind="ExternalInput").ap()
    tT_d = nc.dram_tensor("tT", [TAB, bs], bf, kind="ExternalInput").ap()
    w_fi1_d = nc.dram_tensor("w_fi1", [128, 64 * 128], bf, kind="ExternalInput").ap()
    w_ft1_d = nc.dram_tensor("w_ft1", [128, 4 * 128], bf, kind="ExternalInput").ap()
    w_vr_d = nc.dram_tensor("w_vr", [128, 16 * 128], bf, kind="ExternalInput").ap()
    w_tr_d = nc.dram_tensor("w_tr", [128, 16 * 128], bf, kind="ExternalInput").ap()
    w_ci1_d = nc.dram_tensor("w_ci1", [128, 16, 128], f8, kind="ExternalInput").ap()
    w_ct1_d = nc.dram_tensor("w_ct1", [128, 16, 128], f8, kind="ExternalInput").ap()
    w_vv_d = nc.dram_tensor("w_vv", [128, 16, 128], f8, kind="ExternalInput").ap()
    w_tv_d = nc.dram_tensor("w_tv", [128, 16, 128], f8, kind="ExternalInput").ap()
    bias_d = nc.dram_tensor("bias", [128, 24], f32, kind="ExternalInput").ap()
    out_d = nc.dram_tensor("outT", [2 * HID, bs], bf, kind="ExternalOutput").ap()

    with tile.TileContext(nc) as tc:
        with (
            tc.tile_pool(name="w", bufs=1) as wpool,
            tc.tile_pool(name="x", bufs=16) as xpool,
            tc.tile_pool(name="h", bufs=6) as hpool,
            tc.tile_pool(name="o", bufs=8) as opool,
            tc.tile_pool(name="ps", bufs=8, space="PSUM") as pspool,
        ):
            wf1 = [wpool.tile([128, 4 * 128], bf, name=f"w_fi1_{k}") for k in range(16)]
            wt1 = wpool.tile([128, 4 * 128], bf, name="w_ft1")
            wvr = [wpool.tile([128, 4 * 128], bf, name=f"w_vr_{k}") for k in range(4)]
            wtr = [wpool.tile([128, 4 * 128], bf, name=f"w_tr_{k}") for k in range(4)]
            wc1 = wpool.tile([128, 16, 128], f8, name="w_ci1")
            wc2 = wpool.tile([128, 16, 128], f8, name="w_ct1")
            wvv = wpool.tile([128, 16, 128], f8, name="w_vv")
            wtv = wpool.tile([128, 16, 128], f8, name="w_tv")
            bt = wpool.tile([128, 24], f32, name="bias_t")

            def xload(n):
                xs = []
                c0 = n * NT
                for k in range(16):
                    xk = xpool.tile([128, NT], bf, tag="x", name=f"xk_{n}_{k}")
                    nc.sync.dma_start(xk[:], iT_d[128 * k:128 * (k + 1), c0:c0 + NT])
                    xs.append(xk)
                return xs

            # preamble: first tile's x chunks interleaved with fi1 weight chunks
            x_cur = [xpool.tile([128, NT], bf, tag="x", name=f"xk_0_{k}")
                     for k in range(16)]
            nc.sync.dma_start(bt[:], bias_d[:])
            for k in range(16):
                nc.sync.dma_start(x_cur[k][:], iT_d[128 * k:128 * (k + 1), 0:NT])
                nc.sync.dma_start(wf1[k][:], w_fi1_d[:, 512 * k:512 * (k + 1)])
            xt_cur = xpool.tile([128, NT], bf, tag="xt", bufs=2, name="xt_0")
            nc.sync.dma_start(xt_cur[:], tT_d[:, 0:NT])
            nc.sync.dma_start(wt1[:], w_ft1_d[:])
            nc.sync.dma_start(wc1[:], w_ci1_d[:])
            for j in range(4):
                nc.sync.dma_start(wvr[j][:], w_vr_d[:, 512 * j:512 * (j + 1)])
            nc.sync.dma_start(wvv[:], w_vv_d[:])
            nc.sync.dma_start(wc2[:], w_ct1_d[:])
            for j in range(4):
                nc.sync.dma_start(wtr[j][:], w_tr_d[:, 512 * j:512 * (j + 1)])
            nc.sync.dma_start(wtv[:], w_tv_d[:])

            def mm(ps, wtile, m, x_ap, start, stop):
                nc.tensor.matmul(ps[:], wtile[:, m * 128:(m + 1) * 128], x_ap,
                                 start=start, stop=stop)

            def mm_dr(ps, wtile, k2, m, x3, start, stop):
                idx = (k2 * 4 + m) * 2
                nc.tensor.matmul(ps[:], wtile[:, idx:idx + 2, :], x3[:, :, :],
                                 start=start, stop=stop, perf_mode=DR)

            for n in range(ntiles):
                c0 = n * NT
                # ---- i_ = relu(i @ fi1.T + b), m-outer so ACTs pipeline ----
                i_ = [hpool.tile([128, NT], bf, tag="i_", name=f"i__{n}_{m}")
                      for m in range(4)]
                i8 = [hpool.tile([128, 2, NT], f8, tag="i8", name=f"i8_{n}_{p}")
                      for p in range(2)]
                ps1 = [None] * 4
                ci1_done = False
                for m in range(4):
                    ps1[m] = pspool.tile([128, NT], f32, tag="ps", name=f"ps1_{n}_{m}")
                    for k in range(16):
                        mm(ps1[m], wf1[k], m, x_cur[k][:], k == 0, k == 15)
                        # slot ci1's first DR pair into fi1 m=3's stream; its
                        # i8 pair-0 input finished during m=2
                        if m == 3 and k in (11, 12, 13, 14) and not ci1_done:
                            mdr = k - 11
                            if mdr == 0:
                                ps3 = [pspool.tile([128, NT], f32, tag="ps",
                                                   name=f"ps3_{n}_{q}")
                                       for q in range(4)]
                            mm_dr(ps3[mdr], wc1, 0, mdr, i8[0], True, False)
                            ci1_done = k == 14
                    nc.scalar.activation(i_[m][:], ps1[m][:], Relu,
                                         bias=bt[:, m:m + 1])
                    nc.vector.tensor_scalar_mul(i8[m // 2][:, m % 2, :],
                                                i_[m][:], S_I8)

                # prefetch next tile's inputs (early in Sync program order)
                if n + 1 < ntiles:
                    x_nxt = xload(n + 1)
                    xt_nxt = xpool.tile([128, NT], bf, tag="xt", bufs=2,
                                        name=f"xt_{n + 1}")
                    nc.sync.dma_start(xt_nxt[:], tT_d[:, c0 + NT:c0 + 2 * NT])

                # ---- t_ = relu(t @ ft1.T + b); finish ci1 (k2=1) between ----
                ps2 = [pspool.tile([128, NT], f32, tag="ps", name=f"ps2_{n}_{m}")
                       for m in range(4)]
                t_ = [hpool.tile([128, NT], bf, tag="t_", name=f"t__{n}_{m}")
                      for m in range(4)]
                t8 = [hpool.tile([128, 2, NT], f8, tag="t8", name=f"t8_{n}_{p}")
                      for p in range(2)]
                for m in range(4):
                    mm(ps2[m], wt1, m, xt_cur[:], True, True)
                    mm_dr(ps3[m], wc1, 1, m, i8[1], False, True)
                for m in range(4):
                    nc.scalar.activation(t_[m][:], ps2[m][:], Relu,
                                         bias=bt[:, 4 + m:5 + m])
                    nc.vector.tensor_scalar_mul(t8[m // 2][:, m % 2, :],
                                                t_[m][:], S_T8)

                # ---- v1 = S_V1 * relu(ci1 @ i_ + b)  (fp8 out) ----
                v1 = [hpool.tile([128, 2, NT], f8, tag="v1", name=f"v1_{n}_{p}")
                      for p in range(2)]
                for m in range(4):
                    nc.scalar.activation(v1[m // 2][:, m % 2, :], ps3[m][:], Relu,
                                         scale=S_V1 / (S_I8 * S_W8),
                                         bias=bt[:, 8 + m:9 + m])

                # ---- psV = GRP*(v1 @ WVl.T + i_ @ fi2.T); ct1 rides along ----
                psV = [pspool.tile([128, NT], f32, tag="ps", name=f"psV_{n}_{m}")
                       for m in range(4)]
                ps4 = [pspool.tile([128, NT], f32, tag="ps", name=f"ps4_{n}_{m}")
                       for m in range(4)]
                for m in range(4):
                    mm(psV[m], wvr[0], m, i_[0][:], True, False)
                    mm_dr(psV[m], wvv, 0, m, v1[0], False, False)
                    mm(psV[m], wvr[1], m, i_[1][:], False, False)
                    mm_dr(psV[m], wvv, 1, m, v1[1], False, False)
                    mm(psV[m], wvr[2], m, i_[2][:], False, False)
                    mm_dr(ps4[m], wc2, 0, m, t8[0], True, False)
                    mm(psV[m], wvr[3], m, i_[3][:], False, True)
                    mm_dr(ps4[m], wc2, 1, m, t8[1], False, True)
                for m in range(4):
                    oV = opool.tile([128, NT], bf, tag="o", name=f"oV_{n}_{m}")
                    nc.scalar.activation(oV[:], psV[m][:], Ident,
                                         scale=1.0 / GRP, bias=bt[:, 16 + m:17 + m])
                    nc.sync.dma_start(out_d[128 * m:128 * (m + 1), c0:c0 + NT], oV[:])

                # ---- v2 = S_V2 * relu(ct1 @ t_ + b)  (fp8 out) ----
                v2 = [hpool.tile([128, 2, NT], f8, tag="v2", name=f"v2_{n}_{p}")
                      for p in range(2)]
                for m in range(4):
                    nc.scalar.activation(v2[m // 2][:, m % 2, :], ps4[m][:], Relu,
                                         scale=S_V2 / (S_T8 * S_W8),
                                         bias=bt[:, 12 + m:13 + m])

                # ---- psT = GRP*(v2 @ WTl.T + t_ @ ft2.T) ----
                psT = [pspool.tile([128, NT], f32, tag="ps", name=f"psT_{n}_{m}")
                       for m in range(4)]
                for m in range(4):
                    mm(psT[m], wtr[0], m, t_[0][:], True, False)
                    mm_dr(psT[m], wtv, 0, m, v2[0], False, False)
                    mm(psT[m], wtr[1], m, t_[1][:], False, False)
                    mm_dr(psT[m], wtv, 1, m, v2[1], False, False)
                    mm(psT[m], wtr[2], m, t_[2][:], False, False)
                    mm(psT[m], wtr[3], m, t_[3][:], False, True)
                for m in range(4):
                    oT = opool.tile([128, NT], bf, tag="o", name=f"oT_{n}_{m}")
                    nc.scalar.activation(oT[:], psT[m][:], Ident,
                                         scale=1.0 / GRP, bias=bt[:, 20 + m:21 + m])
                    nc.sync.dma_start(
                        out_d[HID + 128 * m:HID + 128 * (m + 1), c0:c0 + NT], oT[:]
                    )

                if n + 1 < ntiles:
                    x_cur = x_nxt
                    xt_cur = xt_nxt

    nc.compile()
    return nc


def _host_pack(inp: dict):
    f8d = np.float64
    fi1_w, fi1_b = inp["fi1_w"], inp["fi1_b"]
    ft1_w, ft1_b = inp["ft1_w"], inp["ft1_b"]
    ci1_w, ci1_b = inp["ci1_w"], inp["ci1_b"]
    ct1_w, ct1_b = inp["ct1_w"], inp["ct1_b"]

    def fold(wv, bv, wo, bo, f_w, f_b):
        Wvo = wo.astype(f8d) @ wv.astype(f8d)
        bvo = wo.astype(f8d) @ bv.astype(f8d) + bo.astype(f8d)
        Wl = f_w.astype(f8d) @ Wvo                      # [512, 512] for v-path
        bcat = f_w.astype(f8d) @ bvo + f_b.astype(f8d)  # [512]
        return Wl, bcat

    WVl, bcatV = fold(inp["aV_wv"], inp["aV_bv"], inp["aV_wo"], inp["aV_bo"],
                      inp["fi2_w"], inp["fi2_b"])
    WTl, bcatT = fold(inp["aT_wv"], inp["aT_bv"], inp["aT_wo"], inp["aT_bo"],
                      inp["ft2_w"], inp["ft2_b"])

    weights = {
        "w_fi1": _pack_blocks(np.ascontiguousarray(fi1_w.T).astype(f8d), 16, 4),
        "w_ft1": _pack_blocks(np.ascontiguousarray(ft1_w.T).astype(f8d), 1, 4),
        "w_vr": _pack_blocks(np.ascontiguousarray(inp["fi2_w"].T).astype(f8d) * GRP,
                             4, 4),
        "w_tr": _pack_blocks(np.ascontiguousarray(inp["ft2_w"].T).astype(f8d) * GRP,
                             4, 4),
        "w_ci1": _pack_dr(np.ascontiguousarray(ci1_w.T).astype(f8d), S_W8),
        "w_ct1": _pack_dr(np.ascontiguousarray(ct1_w.T).astype(f8d), S_W8),
        "w_vv": _pack_dr(np.ascontiguousarray(WVl.T), S_WV),
        "w_tv": _pack_dr(np.ascontiguousarray(WTl.T), S_WV),
    }
    cols = []
    for b in (fi1_b.astype(f8d), ft1_b.astype(f8d), S_V1 * ci1_b.astype(f8d),
              S_V2 * ct1_b.astype(f8d), bcatV, bcatT):
        for m in range(4):
            cols.append(b[128 * m:128 * (m + 1)])
    weights["bias"] = np.ascontiguousarray(np.stack(cols, axis=1), dtype=np.float32)
    return weights


def _core_maps(inputs: dict, weights: dict):
    i = np.asarray(inputs["i"], dtype=np.float32)
    t = np.asarray(inputs["t"], dtype=np.float32)
    in_maps = []
    for c in range(NCORES):
        sl = slice(c * BS, (c + 1) * BS)
        m = dict(weights)
        m["iT"] = np.ascontiguousarray(i[sl].T).astype(_bf16)
        m["tT"] = np.ascontiguousarray(t[sl].T).astype(_bf16)
        in_maps.append(m)
    return in_maps


def _gather(results) -> np.ndarray:
    out = np.empty((B, 2 * HID), dtype=np.float32)
    for c in range(NCORES):
        out[c * BS:(c + 1) * BS] = results[c]["outT"].astype(np.float32).T
    return out


def kernel(**inputs) -> np.ndarray:
    from concourse import bass_utils

    weights = _host_pack(inputs)

    if "nc" not in _CACHE:
        _CACHE["nc"] = _build_nc(BS)
    nc = _CACHE["nc"]

    in_maps = _core_maps(inputs, weights)
    res = bass_utils.run_bass_kernel_spmd(nc, in_maps, core_ids=list(range(NCORES)))
    return _gather(res.results)


# revision 3
# speedup vs baseline: 1.2704x; 1.0025x over previous
"""Trainium2 Bass kernel for nn_CMFA (dense_transformer, seq_len=1 cross-attention).

Math notes (exact simplifications vs the reference):
  - softmax over a single key is exactly 1.0, so the attention output is
    exactly the v-projection: mha(q,k,v) = (v @ Wv.T + bv) @ Wo.T + bo.
    The q/k projections never influence the output.
  - Wv -> Wo -> fi2 is a linear chain (no nonlinearity), folded on the host:
      V = v1 @ (fi2 @ Wo @ Wv).T + i_ @ fi2.T + bcatV
    (the i_ term carries the residual through fi2), likewise for T.

Precision plan (validated on host against the fp64 reference; gate 2e-2):
  - fi1/ft1 and the residual halves (fi2/ft2) run in bf16 (~4e-3 end to end).
  - The attention-value paths (ci1, ct1 and the v1/v2 halves of the folded
    output matmuls) run in fp8e4m3 with DoubleRow perf mode: a [128,2,128]
    stationary + [128,2,512] moving matmul covers a 256-deep contraction in
    512 cycles (2 MACs/cycle, measured 216 ns/MM warm). Their signal
    contribution is small (wv/wo are 0.02-scale), so fp8 lands at ~5.6e-3.
  - Switching the PE between bf16 and fp8-DR costs ~190 ns per boundary, so
    DR matmuls are emitted in contiguous blocks (4 mode switches per tile).
  - All fp8 operands are pre-scaled by powers of two so that every
    quantization step needs no extra multiply:
      i8 = 0.25*i_, ci1_8 = 128*ci1  -> psum is exactly 32*(i_@ci1.T), and
      v1_8 = relu(psum + 32*b) is produced on the *Vector* engine with a
      single (add, max) op -- no scalar-engine latency in the DR block.
      The bf16 residual weights are scaled by the same 2^17 group factor as
      (32*v1)@(4096*WVl), so fp8-DR and bf16 matmuls share one PSUM group
      and a single output activation applies 2^-17 + bias.

Device layout: activations feature-major ([feat, batch]); pure data parallel
across 8 cores, weights replicated; batch tiles of NT=512, double-buffered
input DMA, outputs stored bf16 and upcast on the host.
"""

import numpy as np
import ml_dtypes

B, IMG, TAB, HID = 32768, 2048, 128, 512
NCORES = 8
BS = B // NCORES  # rows per core
NT = 512          # batch-tile (matmul moving/free dim)

# fp8 scaling (powers of two; e4m3 max-normal on TRN is 240)
S_I8 = 0.25       # i_ scale for the ci1 rhs  (with S_W8: psum = 32*(i_@W))
S_T8 = 0.25
S_W8 = 128.0      # ci1/ct1 weight scale
S_V1 = S_I8 * S_W8  # 32: v1/v2 fp8 scale, free via psum scale
S_WV = 4096.0     # Vv/Tv (folded) weight scale
GRP = S_V1 * S_WV  # 2^17: shared psum group scale for the output matmuls

_CACHE = {}

_bf16 = ml_dtypes.bfloat16
_f8 = ml_dtypes.float8_e4m3  # TRN-style e4m3 (max 240)


def _pack_blocks(WT: np.ndarray, K: int, M: int) -> np.ndarray:
    """[K*128, M*128] -> [128, K*M*128] bf16, block (k,m) at cols (k*M+m)*128."""
    out = WT.reshape(K, 128, M, 128).transpose(1, 0, 2, 3).reshape(128, K * M * 128)
    return np.ascontiguousarray(out).astype(_bf16)


def _pack_dr(WT: np.ndarray, scale: float) -> np.ndarray:
    """[512, 512] -> [128, 16, 128] fp8 for DoubleRow: dim1 = (k2*4+m)*2+kk,
    value = scale*WT[(2*k2+kk)*128+p, m*128+j]."""
    w = (WT * scale).reshape(2, 2, 128, 4, 128)           # k2, kk, p, m, j
    w = w.transpose(2, 0, 3, 1, 4).reshape(128, 16, 128)  # p, (k2,m,kk), j
    return np.clip(np.ascontiguousarray(w), -240, 240).astype(_f8)


def _build_nc(bs: int):
    import concourse.bass as bass  # noqa: F401
    import concourse.tile as tile
    from concourse import bacc, mybir

    f32 = mybir.dt.float32
    bf = mybir.dt.bfloat16
    f8 = mybir.dt.float8e4
    DR = mybir.MatmulPerfMode.DoubleRow
    Relu = mybir.ActivationFunctionType.Relu
    Ident = mybir.ActivationFunctionType.Identity
    ADD = mybir.AluOpType.add
    MAX = mybir.AluOpType.max
    ntiles = bs // NT

    nc = bacc.Bacc("TRN2", target_bir_lowering=False, debug=False)

    iT_d = nc.dram_tensor("iT", [IMG, bs], bf, kind="ExternalInput").ap()
    tT_d = nc.dram_tensor("tT", [TAB, bs], bf, kind="ExternalInput").ap()
    w_fi1_d = nc.dram_tensor("w_fi1", [128, 64 * 128], bf, kind="ExternalInput").ap()
    w_ft1_d = nc.dram_tensor("w_ft1", [128, 4 * 128], bf, kind="ExternalInput").ap()
    w_vr_d = nc.dram_tensor("w_vr", [128, 16 * 128], bf, kind="ExternalInput").ap()
    w_tr_d = nc.dram_tensor("w_tr", [128, 16 * 128], bf, kind="ExternalInput").ap()
    w_ci1_d = nc.dram_tensor("w_ci1", [128, 16, 128], f8, kind="ExternalInput").ap()
    w_ct1_d = nc.dram_tensor("w_ct1", [128, 16, 128], f8, kind="ExternalInput").ap()
    w_vv_d = nc.dram_tensor("w_vv", [128, 16, 128], f8, kind="ExternalInput").ap()
    w_tv_d = nc.dram_tensor("w_tv", [128, 16, 128], f8, kind="ExternalInput").ap()
    bias_d = nc.dram_tensor("bias", [128, 32], f32, kind="ExternalInput").ap()
    out_d = nc.dram_tensor("outT", [2 * HID, bs], bf, kind="ExternalOutput").ap()

    with tile.TileContext(nc) as tc:
        with (
            tc.tile_pool(name="w", bufs=1) as wpool,
            tc.tile_pool(name="x", bufs=16) as xpool,
            tc.tile_pool(name="h", bufs=6) as hpool,
            tc.tile_pool(name="o", bufs=8) as opool,
            tc.tile_pool(name="ps", bufs=8, space="PSUM") as pspool,
        ):
            wf1 = [wpool.tile([128, 4 * 128], bf, name=f"w_fi1_{k}") for k in range(16)]
            wt1 = wpool.tile([128, 4 * 128], bf, name="w_ft1")
            wvr = [wpool.tile([128, 4 * 128], bf, name=f"w_vr_{k}") for k in range(4)]
            wtr = [wpool.tile([128, 4 * 128], bf, name=f"w_tr_{k}") for k in range(4)]
            wc1 = wpool.tile([128, 16, 128], f8, name="w_ci1")
            wc2 = wpool.tile([128, 16, 128], f8, name="w_ct1")
            wvv = wpool.tile([128, 16, 128], f8, name="w_vv")
            wtv = wpool.tile([128, 16, 128], f8, name="w_tv")
            bt = wpool.tile([128, 32], f32, name="bias_t")

            def xload(n):
                xs = []
                c0 = n * NT
                for k in range(16):
                    xk = xpool.tile([128, NT], bf, tag="x", name=f"xk_{n}_{k}")
                    nc.sync.dma_start(xk[:], iT_d[128 * k:128 * (k + 1), c0:c0 + NT])
                    xs.append(xk)
                return xs

            # preamble: first tile's x chunks interleaved with fi1 weight chunks
            x_cur = [xpool.tile([128, NT], bf, tag="x", name=f"xk_0_{k}")
                     for k in range(16)]
            nc.sync.dma_start(bt[:], bias_d[:])
            for k in range(16):
                nc.sync.dma_start(x_cur[k][:], iT_d[128 * k:128 * (k + 1), 0:NT])
                nc.sync.dma_start(wf1[k][:], w_fi1_d[:, 512 * k:512 * (k + 1)])
            xt_cur = xpool.tile([128, NT], bf, tag="xt", bufs=2, name="xt_0")
            nc.sync.dma_start(xt_cur[:], tT_d[:, 0:NT])
            nc.sync.dma_start(wt1[:], w_ft1_d[:])
            nc.sync.dma_start(wc1[:], w_ci1_d[:])
            nc.sync.dma_start(wc2[:], w_ct1_d[:])
            nc.sync.dma_start(wvv[:], w_vv_d[:])
            for j in range(4):
                nc.sync.dma_start(wvr[j][:], w_vr_d[:, 512 * j:512 * (j + 1)])
            nc.sync.dma_start(wtv[:], w_tv_d[:])
            for j in range(4):
                nc.sync.dma_start(wtr[j][:], w_tr_d[:, 512 * j:512 * (j + 1)])

            def mm(ps, wtile, m, x_ap, start, stop):
                nc.tensor.matmul(ps[:], wtile[:, m * 128:(m + 1) * 128], x_ap,
                                 start=start, stop=stop)

            def mm_dr(ps, wtile, k2, m, x3, start, stop):
                idx = (k2 * 4 + m) * 2
                nc.tensor.matmul(ps[:], wtile[:, idx:idx + 2, :], x3[:, :, :],
                                 start=start, stop=stop, perf_mode=DR)

            for n in range(ntiles):
                c0 = n * NT
                # ======== bf16 block: fi1 (m-outer), prefetch, ft1 ========
                i_ = [hpool.tile([128, NT], bf, tag="i_", name=f"i__{n}_{m}")
                      for m in range(4)]
                i8 = [hpool.tile([128, 2, NT], f8, tag="i8", name=f"i8_{n}_{p}")
                      for p in range(2)]
                ps1 = [None] * 4
                for m in range(4):
                    ps1[m] = pspool.tile([128, NT], f32, tag="ps", name=f"ps1_{n}_{m}")
                    for k in range(16):
                        mm(ps1[m], wf1[k], m, x_cur[k][:], k == 0, k == 15)
                    nc.scalar.activation(i8[m // 2][:, m % 2, :], ps1[m][:], Relu,
                                         scale=S_I8, bias=bt[:, 24 + m:25 + m])
                    nc.scalar.activation(i_[m][:], ps1[m][:], Relu,
                                         bias=bt[:, m:m + 1])

                if n + 1 < ntiles:
                    x_nxt = xload(n + 1)
                    xt_nxt = xpool.tile([128, NT], bf, tag="xt", bufs=2,
                                        name=f"xt_{n + 1}")
                    nc.sync.dma_start(xt_nxt[:], tT_d[:, c0 + NT:c0 + 2 * NT])

                ps2 = [pspool.tile([128, NT], f32, tag="ps", name=f"ps2_{n}_{m}")
                       for m in range(4)]
                t_ = [hpool.tile([128, NT], bf, tag="t_", name=f"t__{n}_{m}")
                      for m in range(4)]
                t8 = [hpool.tile([128, 2, NT], f8, tag="t8", name=f"t8_{n}_{p}")
                      for p in range(2)]
                for m in range(4):
                    mm(ps2[m], wt1, m, xt_cur[:], True, True)
                # t8 chunks first: ct1 needs them before Vr needs t_
                for m in range(4):
                    nc.scalar.activation(t8[m // 2][:, m % 2, :], ps2[m][:], Relu,
                                         scale=S_T8, bias=bt[:, 28 + m:29 + m])
                for m in range(4):
                    nc.scalar.activation(t_[m][:], ps2[m][:], Relu,
                                         bias=bt[:, 4 + m:5 + m])

                # ======== DR block 1: ci1 / ct1 / Vv ========
                ps3 = [pspool.tile([128, NT], f32, tag="ps", name=f"ps3_{n}_{m}")
                       for m in range(4)]
                ps4 = [pspool.tile([128, NT], f32, tag="ps", name=f"ps4_{n}_{m}")
                       for m in range(4)]
                for m in range(4):
                    mm_dr(ps3[m], wc1, 0, m, i8[0], True, False)
                for m in range(4):
                    mm_dr(ps4[m], wc2, 0, m, t8[0], True, False)
                for m in range(4):
                    mm_dr(ps3[m], wc1, 1, m, i8[1], False, True)
                for m in range(4):
                    mm_dr(ps4[m], wc2, 1, m, t8[1], False, True)

                # v1/v2 on the Vector engine: fp8 out = max(psum + 32*b, 0)
                v1 = [hpool.tile([128, 2, NT], f8, tag="v1", name=f"v1_{n}_{p}")
                      for p in range(2)]
                v2 = [hpool.tile([128, 2, NT], f8, tag="v2", name=f"v2_{n}_{p}")
                      for p in range(2)]
                for m in range(4):
                    nc.vector.tensor_scalar(v1[m // 2][:, m % 2, :], ps3[m][:],
                                            bt[:, 8 + m:9 + m], 0.0, ADD, MAX)
                for m in range(4):
                    nc.vector.tensor_scalar(v2[m // 2][:, m % 2, :], ps4[m][:],
                                            bt[:, 12 + m:13 + m], 0.0, ADD, MAX)

                psV = [pspool.tile([128, NT], f32, tag="ps", name=f"psV_{n}_{m}")
                       for m in range(4)]
                for k2 in range(2):
                    for m in range(4):
                        mm_dr(psV[m], wvv, k2, m, v1[k2], k2 == 0, False)

                # ======== bf16 block: Vr (+ output V) ========
                for m in range(4):
                    for k in range(4):
                        mm(psV[m], wvr[k], m, i_[k][:], False, k == 3)
                    oV = opool.tile([128, NT], bf, tag="o", name=f"oV_{n}_{m}")
                    nc.scalar.activation(oV[:], psV[m][:], Ident,
                                         scale=1.0 / GRP, bias=bt[:, 16 + m:17 + m])
                    nc.sync.dma_start(out_d[128 * m:128 * (m + 1), c0:c0 + NT], oV[:])

                # ======== DR block 2: Tv ========
                psT = [pspool.tile([128, NT], f32, tag="ps", name=f"psT_{n}_{m}")
                       for m in range(4)]
                for k2 in range(2):
                    for m in range(4):
                        mm_dr(psT[m], wtv, k2, m, v2[k2], k2 == 0, False)

                # ======== bf16 block: Tr (+ output T) ========
                for m in range(4):
                    for k in range(4):
                        mm(psT[m], wtr[k], m, t_[k][:], False, k == 3)
                    oT = opool.tile([128, NT], bf, tag="o", name=f"oT_{n}_{m}")
                    nc.scalar.activation(oT[:], psT[m][:], Ident,
                                         scale=1.0 / GRP, bias=bt[:, 20 + m:21 + m])
                    nc.sync.dma_start(
                        out_d[HID + 128 * m:HID + 128 * (m + 1), c0:c0 + NT], oT[:]
                    )

                if n + 1 < ntiles:
                    x_cur = x_nxt
                    xt_cur = xt_nxt

    nc.compile()
    return nc


def _host_pack(inp: dict):
    f8d = np.float64
    fi1_w, fi1_b = inp["fi1_w"], inp["fi1_b"]
    ft1_w, ft1_b = inp["ft1_w"], inp["ft1_b"]
    ci1_w, ci1_b = inp["ci1_w"], inp["ci1_b"]
    ct1_w, ct1_b = inp["ct1_w"], inp["ct1_b"]

    def fold(wv, bv, wo, bo, f_w, f_b):
        Wvo = wo.astype(f8d) @ wv.astype(f8d)
        bvo = wo.astype(f8d) @ bv.astype(f8d) + bo.astype(f8d)
        Wl = f_w.astype(f8d) @ Wvo                      # [512, 512] for v-path
        bcat = f_w.astype(f8d) @ bvo + f_b.astype(f8d)  # [512]
        return Wl, bcat

    WVl, bcatV = fold(inp["aV_wv"], inp["aV_bv"], inp["aV_wo"], inp["aV_bo"],
                      inp["fi2_w"], inp["fi2_b"])
    WTl, bcatT = fold(inp["aT_wv"], inp["aT_bv"], inp["aT_wo"], inp["aT_bo"],
                      inp["ft2_w"], inp["ft2_b"])

    weights = {
        "w_fi1": _pack_blocks(np.ascontiguousarray(fi1_w.T).astype(f8d), 16, 4),
        "w_ft1": _pack_blocks(np.ascontiguousarray(ft1_w.T).astype(f8d), 1, 4),
        "w_vr": _pack_blocks(np.ascontiguousarray(inp["fi2_w"].T).astype(f8d) * GRP,
                             4, 4),
        "w_tr": _pack_blocks(np.ascontiguousarray(inp["ft2_w"].T).astype(f8d) * GRP,
                             4, 4),
        "w_ci1": _pack_dr(np.ascontiguousarray(ci1_w.T).astype(f8d), S_W8),
        "w_ct1": _pack_dr(np.ascontiguousarray(ct1_w.T).astype(f8d), S_W8),
        "w_vv": _pack_dr(np.ascontiguousarray(WVl.T), S_WV),
        "w_tv": _pack_dr(np.ascontiguousarray(WTl.T), S_WV),
    }
    cols = []
    for b in (fi1_b.astype(f8d), ft1_b.astype(f8d), S_V1 * ci1_b.astype(f8d),
              S_V1 * ct1_b.astype(f8d), bcatV, bcatT,
              S_I8 * fi1_b.astype(f8d), S_T8 * ft1_b.astype(f8d)):
        for m in range(4):
            cols.append(b[128 * m:128 * (m + 1)])
    weights["bias"] = np.ascontiguousarray(np.stack(cols, axis=1), dtype=np.float32)
    return weights


def _core_maps(inputs: dict, weights: dict):
    i = np.asarray(inputs["i"], dtype=np.float32)
    t = np.asarray(inputs["t"], dtype=np.float32)
    in_maps = []
    for c in range(NCORES):
        sl = slice(c * BS, (c + 1) * BS)
        m = dict(weights)
        m["iT"] = np.ascontiguousarray(i[sl].T).astype(_bf16)
        m["tT"] = np.ascontiguousarray(t[sl].T).astype(_bf16)
        in_maps.append(m)
    return in_maps


def _gather(results) -> np.ndarray:
    out = np.empty((B, 2 * HID), dtype=np.float32)
    for c in range(NCORES):
        out[c * BS:(c + 1) * BS] = results[c]["outT"].astype(np.float32).T
    return out


def kernel(**inputs) -> np.ndarray:
    from concourse import bass_utils

    weights = _host_pack(inputs)

    if "nc" not in _CACHE:
        _CACHE["nc"] = _build_nc(BS)
    nc = _CACHE["nc"]

    in_maps = _core_maps(inputs, weights)
    res = bass_utils.run_bass_kernel_spmd(nc, in_maps, core_ids=list(range(NCORES)))
    return _gather(res.results)


# revision 5
# speedup vs baseline: 1.2771x; 1.0053x over previous
"""Trainium2 Bass kernel for nn_CMFA (dense_transformer, seq_len=1 cross-attention).

Math notes (exact simplifications vs the reference):
  - softmax over a single key is exactly 1.0, so the attention output is
    exactly the v-projection: mha(q,k,v) = (v @ Wv.T + bv) @ Wo.T + bo.
    The q/k projections never influence the output.
  - Wv -> Wo -> fi2 is a linear chain (no nonlinearity), folded on the host:
      V = v1 @ (fi2 @ Wo @ Wv).T + i_ @ fi2.T + bcatV
    (the i_ term carries the residual through fi2), likewise for T.

Precision plan (validated on host against the fp64 reference; gate 2e-2):
  - fi1/ft1 and the residual halves (fi2/ft2) run in bf16 (~4e-3 end to end).
  - The attention-value paths (ci1, ct1 and the v1/v2 halves of the folded
    output matmuls) run in fp8e4m3 with DoubleRow perf mode: a [128,2,128]
    stationary + [128,2,512] moving matmul covers a 256-deep contraction in
    512 cycles (2 MACs/cycle, measured 216 ns/MM warm). Their signal
    contribution is small (wv/wo are 0.02-scale), so fp8 lands at ~5.6e-3.
  - Switching the PE between bf16 and fp8-DR costs ~190 ns per boundary, so
    DR matmuls are emitted in contiguous blocks (4 mode switches per tile).
  - All fp8 operands are pre-scaled by powers of two so that every
    quantization step needs no extra multiply:
      i8 = 0.25*i_, ci1_8 = 128*ci1  -> psum is exactly 32*(i_@ci1.T), and
      v1_8 = relu(psum + 32*b) is produced on the *Vector* engine with a
      single (add, max) op -- no scalar-engine latency in the DR block.
      The bf16 residual weights are scaled by the same 2^17 group factor as
      (32*v1)@(4096*WVl), so fp8-DR and bf16 matmuls share one PSUM group
      and a single output activation applies 2^-17 + bias.

Device layout: activations feature-major ([feat, batch]); pure data parallel
across 8 cores, weights replicated; batch tiles of NT=512, double-buffered
input DMA, outputs stored bf16 and upcast on the host.
"""

import numpy as np
import ml_dtypes

B, IMG, TAB, HID = 32768, 2048, 128, 512
NCORES = 8
BS = B // NCORES  # rows per core
NT = 512          # batch-tile (matmul moving/free dim)

# fp8 scaling (powers of two; e4m3 max-normal on TRN is 240)
S_I8 = 0.25       # i_ scale for the ci1 rhs  (with S_W8: psum = 32*(i_@W))
S_T8 = 0.25
S_W8 = 128.0      # ci1/ct1 weight scale
S_V1 = S_I8 * S_W8  # 32: v1/v2 fp8 scale, free via psum scale
S_WV = 4096.0     # Vv/Tv (folded) weight scale
GRP = S_V1 * S_WV  # 2^17: shared psum group scale for the output matmuls

_CACHE = {}

_bf16 = ml_dtypes.bfloat16
_f8 = ml_dtypes.float8_e4m3  # TRN-style e4m3 (max 240)


def _pack_blocks(WT: np.ndarray, K: int, M: int) -> np.ndarray:
    """[K*128, M*128] -> [128, K*M*128] bf16, block (k,m) at cols (k*M+m)*128."""
    out = WT.reshape(K, 128, M, 128).transpose(1, 0, 2, 3).reshape(128, K * M * 128)
    return np.ascontiguousarray(out).astype(_bf16)


def _pack_dr(WT: np.ndarray, scale: float) -> np.ndarray:
    """[512, 512] -> [128, 16, 128] fp8 for DoubleRow: dim1 = (k2*4+m)*2+kk,
    value = scale*WT[(2*k2+kk)*128+p, m*128+j]."""
    w = (WT * scale).reshape(2, 2, 128, 4, 128)           # k2, kk, p, m, j
    w = w.transpose(2, 0, 3, 1, 4).reshape(128, 16, 128)  # p, (k2,m,kk), j
    return np.clip(np.ascontiguousarray(w), -240, 240).astype(_f8)


def _build_nc(bs: int):
    import concourse.bass as bass  # noqa: F401
    import concourse.tile as tile
    from concourse import bacc, mybir

    f32 = mybir.dt.float32
    bf = mybir.dt.bfloat16
    f8 = mybir.dt.float8e4
    DR = mybir.MatmulPerfMode.DoubleRow
    Relu = mybir.ActivationFunctionType.Relu
    Ident = mybir.ActivationFunctionType.Identity
    ADD = mybir.AluOpType.add
    MAX = mybir.AluOpType.max
    ntiles = bs // NT

    nc = bacc.Bacc("TRN2", target_bir_lowering=False, debug=False)

    iT_d = nc.dram_tensor("iT", [IMG, bs], bf, kind="ExternalInput").ap()
    tT_d = nc.dram_tensor("tT", [TAB, bs], bf, kind="ExternalInput").ap()
    w_fi1_d = nc.dram_tensor("w_fi1", [128, 64 * 128], bf, kind="ExternalInput").ap()
    w_ft1_d = nc.dram_tensor("w_ft1", [128, 4 * 128], bf, kind="ExternalInput").ap()
    w_vr_d = nc.dram_tensor("w_vr", [128, 16 * 128], bf, kind="ExternalInput").ap()
    w_tr_d = nc.dram_tensor("w_tr", [128, 16 * 128], bf, kind="ExternalInput").ap()
    w_ci1_d = nc.dram_tensor("w_ci1", [128, 16, 128], f8, kind="ExternalInput").ap()
    w_ct1_d = nc.dram_tensor("w_ct1", [128, 16, 128], f8, kind="ExternalInput").ap()
    w_vv_d = nc.dram_tensor("w_vv", [128, 16, 128], f8, kind="ExternalInput").ap()
    w_tv_d = nc.dram_tensor("w_tv", [128, 16, 128], f8, kind="ExternalInput").ap()
    bias_d = nc.dram_tensor("bias", [128, 32], f32, kind="ExternalInput").ap()
    out_d = nc.dram_tensor("outT", [2 * HID, bs], bf, kind="ExternalOutput").ap()

    with tile.TileContext(nc) as tc:
        with (
            tc.tile_pool(name="w", bufs=1) as wpool,
            tc.tile_pool(name="x", bufs=16) as xpool,
            tc.tile_pool(name="h", bufs=6) as hpool,
            tc.tile_pool(name="o", bufs=8) as opool,
            tc.tile_pool(name="ps", bufs=8, space="PSUM") as pspool,
        ):
            wf1 = [wpool.tile([128, 4 * 128], bf, name=f"w_fi1_{k}") for k in range(16)]
            wt1 = wpool.tile([128, 4 * 128], bf, name="w_ft1")
            wvr = [wpool.tile([128, 4 * 128], bf, name=f"w_vr_{k}") for k in range(4)]
            wtr = [wpool.tile([128, 4 * 128], bf, name=f"w_tr_{k}") for k in range(4)]
            wc1 = wpool.tile([128, 16, 128], f8, name="w_ci1")
            wc2 = wpool.tile([128, 16, 128], f8, name="w_ct1")
            wvv = wpool.tile([128, 16, 128], f8, name="w_vv")
            wtv = wpool.tile([128, 16, 128], f8, name="w_tv")
            bt = wpool.tile([128, 32], f32, name="bias_t")

            def xload(n):
                xs = []
                c0 = n * NT
                for k in range(16):
                    xk = xpool.tile([128, NT], bf, tag="x", name=f"xk_{n}_{k}")
                    nc.sync.dma_start(xk[:], iT_d[128 * k:128 * (k + 1), c0:c0 + NT])
                    xs.append(xk)
                return xs

            # preamble: x chunks on the Sync DMA queue, weights on the Scalar
            # queue (idle during startup) so tile 0 is not DMA-issue paced
            x_cur = [xpool.tile([128, NT], bf, tag="x", name=f"xk_0_{k}")
                     for k in range(16)]
            xt_cur = xpool.tile([128, NT], bf, tag="xt", bufs=2, name="xt_0")
            nc.sync.dma_start(xt_cur[:], tT_d[:, 0:NT])
            nc.scalar.dma_start(bt[:], bias_d[:])
            nc.scalar.dma_start(wt1[:], w_ft1_d[:])
            for k in range(16):
                nc.sync.dma_start(x_cur[k][:], iT_d[128 * k:128 * (k + 1), 0:NT])
                nc.scalar.dma_start(wf1[k][:], w_fi1_d[:, 512 * k:512 * (k + 1)])
            nc.scalar.dma_start(wc1[:], w_ci1_d[:])
            nc.scalar.dma_start(wc2[:], w_ct1_d[:])
            nc.scalar.dma_start(wvv[:], w_vv_d[:])
            for j in range(4):
                nc.scalar.dma_start(wvr[j][:], w_vr_d[:, 512 * j:512 * (j + 1)])
            nc.scalar.dma_start(wtv[:], w_tv_d[:])
            for j in range(4):
                nc.scalar.dma_start(wtr[j][:], w_tr_d[:, 512 * j:512 * (j + 1)])

            def mm(ps, wtile, m, x_ap, start, stop):
                nc.tensor.matmul(ps[:], wtile[:, m * 128:(m + 1) * 128], x_ap,
                                 start=start, stop=stop)

            def mm_dr(ps, wtile, k2, m, x3, start, stop):
                idx = (k2 * 4 + m) * 2
                nc.tensor.matmul(ps[:], wtile[:, idx:idx + 2, :], x3[:, :, :],
                                 start=start, stop=stop, perf_mode=DR)

            for n in range(ntiles):
                c0 = n * NT
                # ======== bf16 block: ft1 first (t-path ACTs hide under fi1),
                # then fi1 (m-outer).  i_/t_ on DVE, i8/t8 on Scalar so each
                # psum bank has one reader per engine queue. ========
                ps2 = [pspool.tile([128, NT], f32, tag="ps", name=f"ps2_{n}_{m}")
                       for m in range(4)]
                t_ = [hpool.tile([128, NT], bf, tag="t_", name=f"t__{n}_{m}")
                      for m in range(4)]
                t8 = [hpool.tile([128, 2, NT], f8, tag="t8", name=f"t8_{n}_{p}")
                      for p in range(2)]
                for m in range(4):
                    mm(ps2[m], wt1, m, xt_cur[:], True, True)
                for m in range(4):
                    nc.scalar.activation(t8[m // 2][:, m % 2, :], ps2[m][:], Relu,
                                         scale=S_T8, bias=bt[:, 28 + m:29 + m])
                    nc.vector.tensor_scalar(t_[m][:], ps2[m][:],
                                            bt[:, 4 + m:5 + m], 0.0, ADD, MAX)

                i_ = [hpool.tile([128, NT], bf, tag="i_", name=f"i__{n}_{m}")
                      for m in range(4)]
                i8 = [hpool.tile([128, 2, NT], f8, tag="i8", name=f"i8_{n}_{p}")
                      for p in range(2)]
                ps1 = [None] * 4
                for m in range(4):
                    ps1[m] = pspool.tile([128, NT], f32, tag="ps", name=f"ps1_{n}_{m}")
                    for k in range(16):
                        mm(ps1[m], wf1[k], m, x_cur[k][:], k == 0, k == 15)
                    nc.scalar.activation(i8[m // 2][:, m % 2, :], ps1[m][:], Relu,
                                         scale=S_I8, bias=bt[:, 24 + m:25 + m])
                    nc.vector.tensor_scalar(i_[m][:], ps1[m][:],
                                            bt[:, m:m + 1], 0.0, ADD, MAX)

                if n + 1 < ntiles:
                    x_nxt = xload(n + 1)
                    xt_nxt = xpool.tile([128, NT], bf, tag="xt", bufs=2,
                                        name=f"xt_{n + 1}")
                    nc.sync.dma_start(xt_nxt[:], tT_d[:, c0 + NT:c0 + 2 * NT])

                # ======== DR block 1: ci1 / ct1 / Vv ========
                ps3 = [pspool.tile([128, NT], f32, tag="ps", name=f"ps3_{n}_{m}")
                       for m in range(4)]
                ps4 = [pspool.tile([128, NT], f32, tag="ps", name=f"ps4_{n}_{m}")
                       for m in range(4)]
                for m in range(4):
                    mm_dr(ps3[m], wc1, 0, m, i8[0], True, False)
                for m in range(4):
                    mm_dr(ps4[m], wc2, 0, m, t8[0], True, False)
                for m in range(4):
                    mm_dr(ps3[m], wc1, 1, m, i8[1], False, True)
                for m in range(4):
                    mm_dr(ps4[m], wc2, 1, m, t8[1], False, True)

                # v1/v2 on the Vector engine: fp8 out = max(psum + 32*b, 0)
                v1 = [hpool.tile([128, 2, NT], f8, tag="v1", name=f"v1_{n}_{p}")
                      for p in range(2)]
                v2 = [hpool.tile([128, 2, NT], f8, tag="v2", name=f"v2_{n}_{p}")
                      for p in range(2)]
                for m in range(4):
                    nc.vector.tensor_scalar(v1[m // 2][:, m % 2, :], ps3[m][:],
                                            bt[:, 8 + m:9 + m], 0.0, ADD, MAX)
                for m in range(4):
                    nc.vector.tensor_scalar(v2[m // 2][:, m % 2, :], ps4[m][:],
                                            bt[:, 12 + m:13 + m], 0.0, ADD, MAX)

                psV = [pspool.tile([128, NT], f32, tag="ps", name=f"psV_{n}_{m}")
                       for m in range(4)]
                for k2 in range(2):
                    for m in range(4):
                        mm_dr(psV[m], wvv, k2, m, v1[k2], k2 == 0, False)

                # ======== bf16 block: Vr (+ output V on the Scalar queue) ====
                for m in range(4):
                    for k in range(4):
                        mm(psV[m], wvr[k], m, i_[k][:], False, k == 3)
                    oV = opool.tile([128, NT], bf, tag="o", name=f"oV_{n}_{m}")
                    nc.scalar.activation(oV[:], psV[m][:], Ident,
                                         scale=1.0 / GRP, bias=bt[:, 16 + m:17 + m])
                    nc.scalar.dma_start(out_d[128 * m:128 * (m + 1), c0:c0 + NT],
                                        oV[:])

                # ======== DR block 2: Tv ========
                psT = [pspool.tile([128, NT], f32, tag="ps", name=f"psT_{n}_{m}")
                       for m in range(4)]
                for k2 in range(2):
                    for m in range(4):
                        mm_dr(psT[m], wtv, k2, m, v2[k2], k2 == 0, False)

                # ======== bf16 block: Tr (+ output T) ========
                for m in range(4):
                    for k in range(4):
                        mm(psT[m], wtr[k], m, t_[k][:], False, k == 3)
                    oT = opool.tile([128, NT], bf, tag="o", name=f"oT_{n}_{m}")
                    nc.scalar.activation(oT[:], psT[m][:], Ident,
                                         scale=1.0 / GRP, bias=bt[:, 20 + m:21 + m])
                    nc.scalar.dma_start(
                        out_d[HID + 128 * m:HID + 128 * (m + 1), c0:c0 + NT], oT[:]
                    )

                if n + 1 < ntiles:
                    x_cur = x_nxt
                    xt_cur = xt_nxt

    nc.compile()
    return nc


def _host_pack(inp: dict):
    f8d = np.float64
    fi1_w, fi1_b = inp["fi1_w"], inp["fi1_b"]
    ft1_w, ft1_b = inp["ft1_w"], inp["ft1_b"]
    ci1_w, ci1_b = inp["ci1_w"], inp["ci1_b"]
    ct1_w, ct1_b = inp["ct1_w"], inp["ct1_b"]

    def fold(wv, bv, wo, bo, f_w, f_b):
        Wvo = wo.astype(f8d) @ wv.astype(f8d)
        bvo = wo.astype(f8d) @ bv.astype(f8d) + bo.astype(f8d)
        Wl = f_w.astype(f8d) @ Wvo                      # [512, 512] for v-path
        bcat = f_w.astype(f8d) @ bvo + f_b.astype(f8d)  # [512]
        return Wl, bcat

    WVl, bcatV = fold(inp["aV_wv"], inp["aV_bv"], inp["aV_wo"], inp["aV_bo"],
                      inp["fi2_w"], inp["fi2_b"])
    WTl, bcatT = fold(inp["aT_wv"], inp["aT_bv"], inp["aT_wo"], inp["aT_bo"],
                      inp["ft2_w"], inp["ft2_b"])

    weights = {
        "w_fi1": _pack_blocks(np.ascontiguousarray(fi1_w.T).astype(f8d), 16, 4),
        "w_ft1": _pack_blocks(np.ascontiguousarray(ft1_w.T).astype(f8d), 1, 4),
        "w_vr": _pack_blocks(np.ascontiguousarray(inp["fi2_w"].T).astype(f8d) * GRP,
                             4, 4),
        "w_tr": _pack_blocks(np.ascontiguousarray(inp["ft2_w"].T).astype(f8d) * GRP,
                             4, 4),
        "w_ci1": _pack_dr(np.ascontiguousarray(ci1_w.T).astype(f8d), S_W8),
        "w_ct1": _pack_dr(np.ascontiguousarray(ct1_w.T).astype(f8d), S_W8),
        "w_vv": _pack_dr(np.ascontiguousarray(WVl.T), S_WV),
        "w_tv": _pack_dr(np.ascontiguousarray(WTl.T), S_WV),
    }
    cols = []
    for b in (fi1_b.astype(f8d), ft1_b.astype(f8d), S_V1 * ci1_b.astype(f8d),
              S_V1 * ct1_b.astype(f8d), bcatV, bcatT,
              S_I8 * fi1_b.astype(f8d), S_T8 * ft1_b.astype(f8d)):
        for m in range(4):
            cols.append(b[128 * m:128 * (m + 1)])
    weights["bias"] = np.ascontiguousarray(np.stack(cols, axis=1), dtype=np.float32)
    return weights


def _core_maps(inputs: dict, weights: dict):
    i = np.asarray(inputs["i"], dtype=np.float32)
    t = np.asarray(inputs["t"], dtype=np.float32)
    in_maps = []
    for c in range(NCORES):
        sl = slice(c * BS, (c + 1) * BS)
        m = dict(weights)
        m["iT"] = np.ascontiguousarray(i[sl].T).astype(_bf16)
        m["tT"] = np.ascontiguousarray(t[sl].T).astype(_bf16)
        in_maps.append(m)
    return in_maps


def _gather(results) -> np.ndarray:
    out = np.empty((B, 2 * HID), dtype=np.float32)
    for c in range(NCORES):
        out[c * BS:(c + 1) * BS] = results[c]["outT"].astype(np.float32).T
    return out


def kernel(**inputs) -> np.ndarray:
    from concourse import bass_utils

    weights = _host_pack(inputs)

    if "nc" not in _CACHE:
        _CACHE["nc"] = _build_nc(BS)
    nc = _CACHE["nc"]

    in_maps = _core_maps(inputs, weights)
    res = bass_utils.run_bass_kernel_spmd(nc, in_maps, core_ids=list(range(NCORES)))
    return _gather(res.results)
